# revision 6
# baseline (speedup 1.0000x reference)
import os
import sys
if '/opt/trn_rl_repo' not in sys.path:
    sys.path.insert(0, '/opt/trn_rl_repo')
import numpy as np
import ml_dtypes

import concourse.bass as bass
import concourse.bacc as bacc
import concourse.mybir as mybir
import concourse.tile as tile
from concourse import library_config
from concourse.masks import make_identity
from concourse.bass_utils import run_bass_kernel_spmd
from concourse._compat import cdiv

NCORE = 8
N_NODES = 100000
N_EDGES = 3200000
NODE_DIM = 128
HID = 16
NGRAPH = 256
NCLS = 3
NLOC = 12544            # nodes per core; 8*12544 = 100352
NT = NLOC // 128        # 98 tiles per core
NTOT = NCORE * NLOC
NQUAD = NTOT // 4
MODS = [("mri", 256), ("cog", 64), ("clin", 32), ("gen", 512)]

F32 = mybir.dt.float32
BF16 = mybir.dt.bfloat16
I16 = mybir.dt.int16
AX = mybir.AxisListType
OP = mybir.AluOpType
ACT = mybir.ActivationFunctionType

RED = os.environ.get('RED_MODE', 'tmult')   # 'tmult' | 'tree'


def _build(Ks):
    """Per-core SPMD program. Ks: unified per-tile max slot counts."""
    Kmax = max(Ks)
    tot4 = 4 * sum(Ks)
    tot8 = 8 * sum(Ks)
    nc = bacc.Bacc(num_swdge_queues=4)
    P = {}
    P['x16'] = nc.declare_dram_parameter("x16", [NLOC, NODE_DIM], BF16, isOutput=False)
    P['W16'] = nc.declare_dram_parameter("W16", [NODE_DIM, HID], BF16, isOutput=False)
    P['dinvT'] = nc.declare_dram_parameter("dinvT", [128, NT], F32, isOutput=False)
    P['brep'] = nc.declare_dram_parameter("brep", [128, HID], F32, isOutput=False)
    P['slotq'] = nc.declare_dram_parameter("slotq", [128, tot8], I16, isOutput=False)
    P['mskv'] = nc.declare_dram_parameter("mskv", [128, tot4], F32, isOutput=False)
    P['Gt'] = nc.declare_dram_parameter("Gt", [128, NT * NGRAPH], BF16, isOutput=False)
    P['cntinv'] = nc.declare_dram_parameter("cntinv", [128, 2], F32, isOutput=False)
    for m, fdim in MODS:
        P[m + 'T'] = nc.declare_dram_parameter(m + 'T', [fdim, NGRAPH], F32, isOutput=False)
        P[m + 'W'] = nc.declare_dram_parameter(m + 'W', [fdim, 4], F32, isOutput=False)
        P[m + 'b'] = nc.declare_dram_parameter(m + 'b', [4, 1], F32, isOutput=False)
    P['cW1'] = nc.declare_dram_parameter("cW1", [32, HID], F32, isOutput=False)
    P['cb1'] = nc.declare_dram_parameter("cb1", [HID, 1], F32, isOutput=False)
    P['cW2'] = nc.declare_dram_parameter("cW2", [HID, NCLS], F32, isOutput=False)
    P['cb2'] = nc.declare_dram_parameter("cb2", [NCLS, 1], F32, isOutput=False)
    out = nc.declare_dram_parameter("out", [NGRAPH, NCLS], F32, isOutput=True)

    tloc = nc.dram_tensor("tloc", [NLOC, HID], F32)
    table = nc.dram_tensor("table", [NTOT, HID], F32, addr_space="Shared")
    poolloc = nc.dram_tensor("poolloc", [HID, NGRAPH], F32)
    poolred = nc.dram_tensor("poolred", [HID, NGRAPH], F32, addr_space="Shared")
    groups = [list(range(NCORE))]

    with tile.TileContext(nc) as tc:
        with tc.tile_pool(name="pers", bufs=1) as pp, \
             tc.tile_pool(name="sb", bufs=3) as sb, \
             tc.tile_pool(name="mskp", bufs=2) as mp_pool, \
             tc.tile_pool(name="gat", bufs=5) as gb, \
             tc.tile_pool(name="ld", bufs=6) as lb, \
             tc.tile_pool(name="ps", bufs=2, space="PSUM") as ps, \
             tc.tile_pool(name="pool_ps", bufs=1, space="PSUM") as pps:
            nc.gpsimd.load_library(library_config.mlp)

            # ---------- phase 1: local xw2 shard + AllGather table ----------
            xT = pp.tile([128, NLOC], BF16)
            nc.sync.dma_start_transpose(xT[:], P['x16'][:])
            Wt = pp.tile([NODE_DIM, HID], BF16)
            nc.sync.dma_start(out=Wt[:], in_=P['W16'][:])
            dinvT = pp.tile([128, NT], F32)
            nc.sync.dma_start(out=dinvT[:], in_=P['dinvT'][:])
            brep = pp.tile([128, HID], F32)
            nc.sync.dma_start(out=brep[:], in_=P['brep'][:])
            ident = pp.tile([128, 128], F32)
            make_identity(nc, ident[:])
            selfb = pp.tile([128, NT * HID], F32)

            for t in range(NT):
                xwp = ps.tile([128, HID], F32, tag="smallps")
                nc.tensor.matmul(xwp[:], xT[:, t * 128:(t + 1) * 128], Wt[:],
                                 start=True, stop=True)
                xw2 = sb.tile([128, HID], F32, tag="xw2")
                nc.vector.tensor_tensor(out=xw2[:], in0=xwp[:],
                                        in1=dinvT[:, t:t + 1].to_broadcast([128, HID]),
                                        op=OP.mult)
                nc.sync.dma_start(out=tloc[t * 128:(t + 1) * 128, :], in_=xw2[:])
                sbt = sb.tile([128, HID], F32, tag="sbt")
                nc.vector.tensor_tensor(out=sbt[:], in0=xw2[:],
                                        in1=dinvT[:, t:t + 1].to_broadcast([128, HID]),
                                        op=OP.mult)
                nc.vector.tensor_add(selfb[:, t * HID:(t + 1) * HID], sbt[:], brep[:])

            nc.gpsimd.collective_compute(
                "AllGather", OP.bypass, replica_groups=groups,
                ins=[tloc[:]], outs=[table[:]])

            # ---------- phase 2: gather + mask-scale + reduce + pool ----------
            tview = table[:].rearrange("(q s) f -> q (s f)", s=4)
            pool_psum = pps.tile([HID, NGRAPH], F32)
            woff = 0
            moff = 0
            for t in range(NT):
                K = Ks[t]
                J = 4 * K
                it = lb.tile([128, 8 * Kmax], I16, tag="it")
                nc.sync.dma_start(out=it[:, :8 * K], in_=P['slotq'][:, woff:woff + 8 * K])
                mk = lb.tile([128, 4 * Kmax], F32, tag="mk")
                nc.sync.dma_start(out=mk[:, :4 * K], in_=P['mskv'][:, moff:moff + 4 * K])
                gtile = lb.tile([128, NGRAPH], BF16, tag="gtile")
                nc.sync.dma_start(out=gtile[:],
                                  in_=P['Gt'][:, t * NGRAPH:(t + 1) * NGRAPH])
                woff += 8 * K
                moff += 4 * K

                gt = gb.tile([128, Kmax * 64], F32, tag="gt")
                nc.gpsimd.dma_gather(
                    gt[:, :K * 64].rearrange("p (c e) -> p c e", e=64),
                    tview, it[:, :8 * K], 128 * K, 128 * K, 64,
                    single_packet=False, queue_num=t % 4)

                nsum = sb.tile([128, HID], F32, tag="nsum")
                if RED == 'tmult':
                    mskd = mp_pool.tile([128, Kmax * 64], F32, tag="mskd")
                    # write transposed (f-major) so the reduce is contiguous
                    nc.vector.tensor_tensor(
                        out=mskd[:, :J * HID].rearrange("p (f j) -> p j f", f=HID),
                        in0=gt[:, :K * 64].rearrange("p (j f) -> p j f", f=HID),
                        in1=mk[:, :J].unsqueeze(2).to_broadcast([128, J, HID]),
                        op=OP.mult)
                    nc.vector.tensor_reduce(
                        nsum[:], mskd[:, :J * HID].rearrange("p (f j) -> p f j", f=HID),
                        axis=AX.X, op=OP.add)
                else:  # tree
                    mskd = mp_pool.tile([128, Kmax * 64], F32, tag="mskd")
                    nc.vector.tensor_tensor(
                        out=mskd[:, :J * HID].rearrange("p (j f) -> p j f", f=HID),
                        in0=gt[:, :K * 64].rearrange("p (j f) -> p j f", f=HID),
                        in1=mk[:, :J].unsqueeze(2).to_broadcast([128, J, HID]),
                        op=OP.mult)
                    n = J
                    while n > 4:
                        h = n // 2
                        odd = n - 2 * h
                        nc.vector.tensor_add(
                            mskd[:, :h * HID], mskd[:, :h * HID],
                            mskd[:, h * HID:2 * h * HID])
                        if odd:
                            nc.vector.tensor_add(
                                mskd[:, :HID], mskd[:, :HID],
                                mskd[:, 2 * h * HID:n * HID])
                        n = h
                    nc.vector.tensor_reduce(
                        nsum[:], mskd[:, :n * HID].rearrange("p (j f) -> p f j", f=HID),
                        axis=AX.X, op=OP.add)

                pre = sb.tile([128, HID], F32, tag="pre")
                nc.vector.tensor_add(pre[:], nsum[:], selfb[:, t * HID:(t + 1) * HID])
                f17 = sb.tile([128, HID], BF16, tag="f17")
                nc.scalar.activation(f17[:], pre[:], ACT.Relu)
                nc.tensor.matmul(pool_psum[:], f17[:], gtile[:],
                                 start=(t == 0), stop=(t == NT - 1))

            # ---------- phase 3: all-reduce pooled sums; replicated head ----------
            pool_s = sb.tile([HID, NGRAPH], F32)
            nc.vector.tensor_copy(pool_s[:], pool_psum[:])
            nc.sync.dma_start(out=poolloc[:], in_=pool_s[:])
            nc.gpsimd.collective_compute(
                "AllReduce", OP.add, replica_groups=groups,
                ins=[poolloc[:]], outs=[poolred[:]])
            pool_r = pp.tile([HID, NGRAPH], F32)
            nc.sync.dma_start(out=pool_r[:], in_=poolred[:])
            cntinv = pp.tile([128, 2], F32)
            nc.sync.dma_start(out=cntinv[:], in_=P['cntinv'][:])

            # modality MLPs -> [4, 256] tiles in sbuf
            mod_sb = {}
            for m, fdim in MODS:
                mt = pp.tile([fdim if fdim <= 128 else 128,
                              NGRAPH * cdiv(fdim, 128)], F32, tag="mt_" + m)
                if fdim <= 128:
                    nc.sync.dma_start(out=mt[:fdim, :NGRAPH], in_=P[m + 'T'][:])
                else:
                    for k in range(fdim // 128):
                        nc.sync.dma_start(out=mt[:, k * NGRAPH:(k + 1) * NGRAPH],
                                          in_=P[m + 'T'][k * 128:(k + 1) * 128, :])
                wt = pp.tile([fdim if fdim <= 128 else 128,
                              4 * cdiv(fdim, 128)], F32, tag="mw_" + m)
                if fdim <= 128:
                    nc.sync.dma_start(out=wt[:fdim, :4], in_=P[m + 'W'][:])
                else:
                    for k in range(fdim // 128):
                        nc.sync.dma_start(out=wt[:, k * 4:(k + 1) * 4],
                                          in_=P[m + 'W'][k * 128:(k + 1) * 128, :])
                bt = pp.tile([4, 1], F32, tag="mb_" + m)
                nc.sync.dma_start(out=bt[:], in_=P[m + 'b'][:])
                mp = ps.tile([4, NGRAPH], F32, tag="smallps")
                nk = cdiv(fdim, 128)
                for k in range(nk):
                    kk = min(128, fdim - k * 128)
                    nc.tensor.matmul(mp[:], wt[:kk, k * 4:k * 4 + 4],
                                     mt[:kk, k * NGRAPH:(k + 1) * NGRAPH],
                                     start=(k == 0), stop=(k == nk - 1))
                msb = pp.tile([4, NGRAPH], F32, tag="msb_" + m)
                nc.scalar.activation(msb[:], mp[:], ACT.Relu, bias=bt[:])
                mod_sb[m] = msb

            cW1 = pp.tile([32, HID], F32)
            nc.sync.dma_start(out=cW1[:], in_=P['cW1'][:])
            cb1 = pp.tile([HID, 1], F32)
            nc.sync.dma_start(out=cb1[:], in_=P['cb1'][:])
            cW2 = pp.tile([HID, NCLS], F32)
            nc.sync.dma_start(out=cW2[:], in_=P['cW2'][:])
            cb2 = pp.tile([NCLS, 1], F32)
            nc.sync.dma_start(out=cb2[:], in_=P['cb2'][:])

            for gtl in range(2):
                gsl = slice(gtl * 128, (gtl + 1) * 128)
                tp = ps.tile([128, HID], F32, tag="smallps")
                nc.tensor.transpose(tp[:], pool_r[:, gsl], ident[:HID, :HID])
                comb = sb.tile([128, 32], F32, tag="comb")
                nc.vector.tensor_scalar_mul(comb[:, :HID], tp[:],
                                            cntinv[:, gtl:gtl + 1])
                coff = HID
                for m, fdim in MODS:
                    mtp = ps.tile([128, 4], F32, tag="smallps")
                    nc.tensor.transpose(mtp[:], mod_sb[m][:, gsl], ident[:4, :4])
                    nc.vector.tensor_copy(comb[:, coff:coff + 4], mtp[:])
                    coff += 4
                ctp = ps.tile([32, 128], F32, tag="smallps")
                nc.tensor.transpose(ctp[:], comb[:], ident[:])
                combT = sb.tile([32, 128], F32, tag="combT")
                nc.vector.tensor_copy(combT[:], ctp[:])
                hp = ps.tile([HID, 128], F32, tag="smallps")
                nc.tensor.matmul(hp[:], cW1[:], combT[:], start=True, stop=True)
                hT = sb.tile([HID, 128], F32, tag="hT")
                nc.scalar.activation(hT[:], hp[:], ACT.Relu, bias=cb1[:])
                lp = ps.tile([NCLS, 128], F32, tag="smallps")
                nc.tensor.matmul(lp[:], cW2[:], hT[:], start=True, stop=True)
                lT = sb.tile([NCLS, 128], F32, tag="lT")
                nc.vector.tensor_scalar_add(lT[:], lp[:], cb2[:])
                ltp = ps.tile([128, NCLS], F32, tag="smallps")
                nc.tensor.transpose(ltp[:], lT[:], ident[:NCLS, :NCLS])
                lg = sb.tile([128, NCLS], F32, tag="lg")
                nc.vector.tensor_copy(lg[:], ltp[:])
                mx = sb.tile([128, 1], F32, tag="mx")
                nc.vector.tensor_reduce(mx[:], lg[:], axis=AX.X, op=OP.max)
                sh = sb.tile([128, NCLS], F32, tag="sh")
                nc.vector.tensor_scalar_sub(sh[:], lg[:], mx[:])
                ex = sb.tile([128, NCLS], F32, tag="ex")
                nc.scalar.activation(ex[:], sh[:], ACT.Exp)
                sm = sb.tile([128, 1], F32, tag="sm")
                nc.vector.tensor_reduce(sm[:], ex[:], axis=AX.X, op=OP.add)
                lns = sb.tile([128, 1], F32, tag="lns")
                nc.scalar.activation(lns[:], sm[:], ACT.Ln)
                fin = sb.tile([128, NCLS], F32, tag="fin")
                nc.vector.tensor_scalar_sub(fin[:], sh[:], lns[:])
                nc.sync.dma_start(out=out[gsl, :], in_=fin[:])
    nc.compile()
    return nc


def kernel(x, edge_index, batch, mri, cog, clin, genetic,
           gcn_W, gcn_b, mri_W, mri_b, cog_W, cog_b, clin_W, clin_b,
           gen_W, gen_b, cls_W1, cls_b1, cls_W2, cls_b2):
    x = np.asarray(x, np.float32)
    src = np.asarray(edge_index[0], np.int64)
    dst = np.asarray(edge_index[1], np.int64)
    batch = np.asarray(batch, np.int64)

    deg_in = np.bincount(dst, minlength=NTOT)
    dinv = 1.0 / np.sqrt(deg_in + 1.0)       # deg incl. self loop
    is_real = (np.arange(NTOT) < N_NODES).astype(np.int8)

    # global rank: pads first, then ascending in-degree -> core r%8, pos r//8
    ranked = np.lexsort((deg_in, is_real))    # node ids in rank order
    rank = np.empty(NTOT, np.int64)
    rank[ranked] = np.arange(NTOT)
    core_of = rank % NCORE
    posc = rank // NCORE
    gpos = core_of * NLOC + posc              # table row of each node
    squad_all = (gpos[src] // 4).astype(np.int16)
    ssub_all = (gpos[src] % 4).astype(np.int64)
    dcore = core_of[dst]
    dl_all = posc[dst]

    # unified per-tile K
    Ks = np.ones(NT, np.int64)
    cnts_by_core = []
    for c in range(NCORE):
        sel = dcore == c
        cnts = np.bincount(dl_all[sel], minlength=NLOC)
        cnts_by_core.append(cnts)
        Ks = np.maximum(Ks, cnts.reshape(NT, 128).max(axis=1))
    Ks = [int(k) for k in Ks]
    cum8 = np.zeros(NT + 1, np.int64)
    np.cumsum([8 * k for k in Ks], out=cum8[1:])
    cum4 = np.zeros(NT + 1, np.int64)
    np.cumsum([4 * k for k in Ks], out=cum4[1:])
    tot8, tot4 = int(cum8[-1]), int(cum4[-1])

    counts_g = np.bincount(batch, minlength=NGRAPH).astype(np.float32)
    cntinv = (1.0 / np.maximum(counts_g, 1.0)).reshape(2, 128).T.copy()

    giota = np.arange(NGRAPH)
    in_maps = []
    for c in range(NCORE):
        sel = np.nonzero(dcore == c)[0]
        dl = dl_all[sel]
        order = np.argsort(dl, kind='stable')
        dl_s = dl[order]
        sq_s = squad_all[sel][order]
        ss_s = ssub_all[sel][order]
        cnts = cnts_by_core[c]
        starts = np.zeros(NLOC + 1, np.int64)
        np.cumsum(cnts, out=starts[1:])
        kk = np.arange(len(dl_s)) - starts[dl_s]
        tt = dl_s // 128
        ppart = dl_s % 128
        Ks_arr = np.asarray(Ks, np.int64)

        # slot stream index j = k*128 + p within tile t
        j = kk * 128 + ppart
        # idx wrap: partition j%16, free col cum8[t] + j//16
        slotq16 = np.zeros((16, tot8), np.int16)
        slotq16[j % 16, cum8[tt] + j // 16] = sq_s
        slotq = np.tile(slotq16, (8, 1))
        # msk values at [p, cum4[t] + k*4 + sub]
        mskv = np.zeros((128, tot4), np.float32)
        mskv[ppart, cum4[tt] + kk * 4 + ss_s] = dinv[dst[sel][order]].astype(np.float32)

        node_at = ranked[c::NCORE]            # node at each position
        realm = node_at < N_NODES
        xs = np.zeros((NLOC, NODE_DIM), np.float32)
        xs[realm] = x[node_at[realm]]
        gid = np.where(realm, batch[np.minimum(node_at, N_NODES - 1)], -1)
        Gt = np.zeros((128, NT * NGRAPH), ml_dtypes.bfloat16)
        posn = np.arange(NLOC)
        rp = posn[realm]
        Gt[rp % 128, (rp // 128) * NGRAPH + gid[realm]] = 1.0

        m = {
            'x16': xs.astype(ml_dtypes.bfloat16),
            'W16': np.asarray(gcn_W, np.float32).astype(ml_dtypes.bfloat16),
            'dinvT': dinv[node_at].reshape(NT, 128).T.astype(np.float32).copy(),
            'brep': np.tile(np.asarray(gcn_b, np.float32), (128, 1)),
            'slotq': slotq, 'mskv': mskv, 'Gt': Gt, 'cntinv': cntinv,
            'cW1': np.asarray(cls_W1, np.float32),
            'cb1': np.asarray(cls_b1, np.float32).reshape(-1, 1),
            'cW2': np.asarray(cls_W2, np.float32),
            'cb2': np.asarray(cls_b2, np.float32).reshape(-1, 1),
        }
        for (mn, fdim), mv, wv, bv in zip(
                MODS, (mri, cog, clin, genetic),
                (mri_W, cog_W, clin_W, gen_W), (mri_b, cog_b, clin_b, gen_b)):
            m[mn + 'T'] = np.asarray(mv, np.float32).T.copy()
            m[mn + 'W'] = np.asarray(wv, np.float32)
            m[mn + 'b'] = np.asarray(bv, np.float32).reshape(-1, 1)
        in_maps.append(m)

    nc = _build(Ks)
    res = run_bass_kernel_spmd(nc, in_maps, core_ids=list(range(NCORE)))
    return res.results[0]["out"].astype(np.float32)


# revision 8
# speedup vs baseline: 1.2305x; 1.2305x over previous
import sys
if '/opt/trn_rl_repo' not in sys.path:
    sys.path.insert(0, '/opt/trn_rl_repo')
import numpy as np
import ml_dtypes

import concourse.bass as bass
import concourse.bacc as bacc
import concourse.mybir as mybir
import concourse.tile as tile
from concourse import library_config
from concourse.masks import make_identity
from concourse.bass_utils import run_bass_kernel_spmd
from concourse._compat import cdiv

NCORE = 8
N_NODES = 100000
N_EDGES = 3200000
NODE_DIM = 128
HID = 16
NGRAPH = 256
NCLS = 3
NLOC = 12544            # nodes per core (98 tiles of 128); 8*12544 = 100352
NT = NLOC // 128        # 98 tiles per core
NTOT = NCORE * NLOC
MODS = [("mri", 256), ("cog", 64), ("clin", 32), ("gen", 512)]

F32 = mybir.dt.float32
BF16 = mybir.dt.bfloat16
I16 = mybir.dt.int16
AX = mybir.AxisListType
OP = mybir.AluOpType
ACT = mybir.ActivationFunctionType


def _build(Ks, totw, tot4):
    """Per-core SPMD program; same pipeline shape as the proven baseline,
    with the sub-slot select folded into a host-built mask (msk carries
    dinv[dst] at the live sub-slot, 0 elsewhere)."""
    nc = bacc.Bacc(num_swdge_queues=4)
    P = {}
    P['x16'] = nc.declare_dram_parameter("x16", [NLOC, NODE_DIM], BF16, isOutput=False)
    P['W16'] = nc.declare_dram_parameter("W16", [NODE_DIM, HID], BF16, isOutput=False)
    P['degT'] = nc.declare_dram_parameter("degT", [128, NT], F32, isOutput=False)
    P['gidxT'] = nc.declare_dram_parameter("gidxT", [128, NT], F32, isOutput=False)
    P['giota'] = nc.declare_dram_parameter("giota", [128, NGRAPH], F32, isOutput=False)
    P['brep'] = nc.declare_dram_parameter("brep", [128, HID], F32, isOutput=False)
    P['slotq'] = nc.declare_dram_parameter("slotq", [128, totw], I16, isOutput=False)
    P['mskv'] = nc.declare_dram_parameter("mskv", [128, tot4], F32, isOutput=False)
    for m, fdim in MODS:
        P[m + 'T'] = nc.declare_dram_parameter(m + 'T', [fdim, NGRAPH], F32, isOutput=False)
        P[m + 'W'] = nc.declare_dram_parameter(m + 'W', [fdim, 4], F32, isOutput=False)
        P[m + 'b'] = nc.declare_dram_parameter(m + 'b', [4, 1], F32, isOutput=False)
    P['cW1'] = nc.declare_dram_parameter("cW1", [32, HID], F32, isOutput=False)
    P['cb1'] = nc.declare_dram_parameter("cb1", [HID, 1], F32, isOutput=False)
    P['cW2'] = nc.declare_dram_parameter("cW2", [HID, NCLS], F32, isOutput=False)
    P['cb2'] = nc.declare_dram_parameter("cb2", [NCLS, 1], F32, isOutput=False)
    out = nc.declare_dram_parameter("out", [NGRAPH, NCLS], F32, isOutput=True)

    tloc = nc.dram_tensor("tloc", [NLOC, HID], F32)
    table = nc.dram_tensor("table", [NTOT, HID], F32, addr_space="Shared")
    poolloc = nc.dram_tensor("poolloc", [HID + 1, NGRAPH], F32)
    poolred = nc.dram_tensor("poolred", [HID + 1, NGRAPH], F32, addr_space="Shared")
    groups = [list(range(NCORE))]

    with tile.TileContext(nc) as tc:
        with tc.tile_pool(name="pers", bufs=1) as pp, \
             tc.tile_pool(name="sb", bufs=2) as sb, \
             tc.tile_pool(name="gat", bufs=6) as gb, \
             tc.tile_pool(name="ps", bufs=2, space="PSUM") as ps, \
             tc.tile_pool(name="pool_ps", bufs=1, space="PSUM") as pps:
            nc.gpsimd.load_library(library_config.mlp)

            # ---------- phase 1: local xw2 shard + AllGather table ----------
            xT = pp.tile([128, NLOC], BF16)
            nc.sync.dma_start_transpose(xT[:], P['x16'][:])
            Wt = pp.tile([NODE_DIM, HID], BF16)
            nc.sync.dma_start(out=Wt[:], in_=P['W16'][:])
            degT = pp.tile([128, NT], F32)
            nc.sync.dma_start(out=degT[:], in_=P['degT'][:])
            gidxT = pp.tile([128, NT], F32)
            nc.sync.dma_start(out=gidxT[:], in_=P['gidxT'][:])
            giota = pp.tile([128, NGRAPH], F32)
            nc.sync.dma_start(out=giota[:], in_=P['giota'][:])
            brep = pp.tile([128, HID], F32)
            nc.sync.dma_start(out=brep[:], in_=P['brep'][:])
            ident = pp.tile([128, 128], F32)
            make_identity(nc, ident[:])
            ones_t = pp.tile([128, 1], BF16)
            nc.vector.memset(ones_t[:], 1.0)

            dinvT = pp.tile([128, NT], F32)
            rcpT = pp.tile([128, NT], F32)
            nc.vector.reciprocal(rcpT[:], degT[:])
            nc.scalar.activation(dinvT[:], rcpT[:], ACT.Sqrt)
            selfb = pp.tile([128, NT * HID], F32)

            for t in range(NT):
                xwp = ps.tile([128, HID], F32, tag="smallps")
                nc.tensor.matmul(xwp[:], xT[:, t * 128:(t + 1) * 128], Wt[:],
                                 start=True, stop=True)
                xw2 = sb.tile([128, HID], F32, tag="xw2")
                nc.vector.tensor_tensor(out=xw2[:], in0=xwp[:],
                                        in1=dinvT[:, t:t + 1].to_broadcast([128, HID]),
                                        op=OP.mult)
                nc.sync.dma_start(out=tloc[t * 128:(t + 1) * 128, :], in_=xw2[:])
                sbt = sb.tile([128, HID], F32, tag="sbt")
                nc.vector.tensor_tensor(out=sbt[:], in0=xw2[:],
                                        in1=dinvT[:, t:t + 1].to_broadcast([128, HID]),
                                        op=OP.mult)
                nc.vector.tensor_add(selfb[:, t * HID:(t + 1) * HID], sbt[:], brep[:])

            nc.gpsimd.collective_compute(
                "AllGather", OP.bypass, replica_groups=groups,
                ins=[tloc[:]], outs=[table[:]])

            # ---------- phase 2: gather + masked aggregate + pool ----------
            tview = table[:].rearrange("(q s) f -> q (s f)", s=4)
            pool_psum = pps.tile([HID + 1, NGRAPH], F32)
            woff = 0
            ooff = 0
            for t in range(NT):
                K = Ks[t]
                it = gb.tile([128, 8 * max(Ks)], I16, tag="it")
                nc.sync.dma_start(out=it[:, :8 * K], in_=P['slotq'][:, woff:woff + 8 * K])
                mk = gb.tile([128, 4 * max(Ks)], F32, tag="mk")
                nc.sync.dma_start(out=mk[:, :4 * K], in_=P['mskv'][:, ooff:ooff + 4 * K])
                woff += 8 * K
                ooff += 4 * K

                gt = gb.tile([128, max(Ks) * 64], F32, tag="gt")
                nc.gpsimd.dma_gather(
                    gt[:, :K * 64].rearrange("p (c e) -> p c e", e=64),
                    tview, it[:, :8 * K], 128 * K, 128 * K, 64,
                    single_packet=False, queue_num=t % 4)
                mskd = sb.tile([128, max(Ks) * 64], BF16, tag="mskd")
                nc.vector.tensor_tensor(
                    out=mskd[:, :K * 64].rearrange("p (m f) -> p m f", f=HID),
                    in0=gt[:, :K * 64].rearrange("p (m f) -> p m f", f=HID),
                    in1=mk[:, :4 * K].unsqueeze(2).to_broadcast([128, 4 * K, HID]),
                    op=OP.mult)
                nsum = sb.tile([128, HID], F32, tag="nsum")
                nc.vector.tensor_reduce(
                    nsum[:], mskd[:, :K * 64].rearrange("p (m f) -> p f m", f=HID),
                    axis=AX.X, op=OP.add)

                pre = sb.tile([128, HID], F32, tag="pre")
                nc.vector.tensor_add(pre[:], nsum[:], selfb[:, t * HID:(t + 1) * HID])
                f17 = sb.tile([128, HID + 1], BF16, tag="f17")
                nc.scalar.activation(f17[:, :HID], pre[:], ACT.Relu)
                nc.vector.tensor_copy(f17[:, HID:HID + 1], ones_t[:])
                G = sb.tile([128, NGRAPH], BF16, tag="G")
                nc.vector.tensor_tensor(out=G[:], in0=gidxT[:, t:t + 1].to_broadcast([128, NGRAPH]),
                                        in1=giota[:], op=OP.is_equal)
                nc.tensor.matmul(pool_psum[:], f17[:], G[:],
                                 start=(t == 0), stop=(t == NT - 1))

            # ---------- phase 3: all-reduce pooled sums; replicated head ----------
            pool_s = sb.tile([HID + 1, NGRAPH], F32)
            nc.vector.tensor_copy(pool_s[:], pool_psum[:])
            nc.sync.dma_start(out=poolloc[:], in_=pool_s[:])
            nc.gpsimd.collective_compute(
                "AllReduce", OP.add, replica_groups=groups,
                ins=[poolloc[:]], outs=[poolred[:]])
            pool_r = pp.tile([HID + 1, NGRAPH], F32)
            nc.sync.dma_start(out=pool_r[:], in_=poolred[:])

            # modality MLPs -> mT tiles [4, 256] in sbuf
            mod_sb = {}
            for m, fdim in MODS:
                mt = pp.tile([fdim if fdim <= 128 else 128,
                              NGRAPH * cdiv(fdim, 128)], F32, tag="mt_" + m)
                if fdim <= 128:
                    nc.sync.dma_start(out=mt[:fdim, :NGRAPH], in_=P[m + 'T'][:])
                else:
                    for k in range(fdim // 128):
                        nc.sync.dma_start(out=mt[:, k * NGRAPH:(k + 1) * NGRAPH],
                                          in_=P[m + 'T'][k * 128:(k + 1) * 128, :])
                wt = pp.tile([fdim if fdim <= 128 else 128,
                              4 * cdiv(fdim, 128)], F32, tag="mw_" + m)
                if fdim <= 128:
                    nc.sync.dma_start(out=wt[:fdim, :4], in_=P[m + 'W'][:])
                else:
                    for k in range(fdim // 128):
                        nc.sync.dma_start(out=wt[:, k * 4:(k + 1) * 4],
                                          in_=P[m + 'W'][k * 128:(k + 1) * 128, :])
                bt = pp.tile([4, 1], F32, tag="mb_" + m)
                nc.sync.dma_start(out=bt[:], in_=P[m + 'b'][:])
                mp = ps.tile([4, NGRAPH], F32, tag="smallps")
                nk = cdiv(fdim, 128)
                for k in range(nk):
                    kk = min(128, fdim - k * 128)
                    nc.tensor.matmul(mp[:], wt[:kk, k * 4:k * 4 + 4],
                                     mt[:kk, k * NGRAPH:(k + 1) * NGRAPH],
                                     start=(k == 0), stop=(k == nk - 1))
                msb = pp.tile([4, NGRAPH], F32, tag="msb_" + m)
                nc.scalar.activation(msb[:], mp[:], ACT.Relu, bias=bt[:])
                mod_sb[m] = msb

            cW1 = pp.tile([32, HID], F32)
            nc.sync.dma_start(out=cW1[:], in_=P['cW1'][:])
            cb1 = pp.tile([HID, 1], F32)
            nc.sync.dma_start(out=cb1[:], in_=P['cb1'][:])
            cW2 = pp.tile([HID, NCLS], F32)
            nc.sync.dma_start(out=cW2[:], in_=P['cW2'][:])
            cb2 = pp.tile([NCLS, 1], F32)
            nc.sync.dma_start(out=cb2[:], in_=P['cb2'][:])

            for gtile in range(2):
                gsl = slice(gtile * 128, (gtile + 1) * 128)
                # transpose pooled sums -> [128 graphs, 17]
                tp = ps.tile([128, HID + 1], F32, tag="smallps")
                nc.tensor.transpose(tp[:], pool_r[:, gsl], ident[:HID + 1, :HID + 1])
                gsum = sb.tile([128, HID + 1], F32, tag="gsum")
                nc.vector.tensor_copy(gsum[:], tp[:])
                cnt = sb.tile([128, 1], F32, tag="cnt")
                nc.vector.tensor_scalar_max(cnt[:], gsum[:, HID:HID + 1], 1.0)
                rec = sb.tile([128, 1], F32, tag="rec")
                nc.vector.reciprocal(rec[:], cnt[:])
                comb = sb.tile([128, 32], F32, tag="comb")
                nc.vector.tensor_scalar_mul(comb[:, :HID], gsum[:, :HID], rec[:])
                coff = HID
                for m, fdim in MODS:
                    mtp = ps.tile([128, 4], F32, tag="smallps")
                    nc.tensor.transpose(mtp[:], mod_sb[m][:, gsl], ident[:4, :4])
                    nc.vector.tensor_copy(comb[:, coff:coff + 4], mtp[:])
                    coff += 4
                # classifier
                ctp = ps.tile([32, 128], F32, tag="smallps")
                nc.tensor.transpose(ctp[:], comb[:], ident[:])
                combT = sb.tile([32, 128], F32, tag="combT")
                nc.vector.tensor_copy(combT[:], ctp[:])
                hp = ps.tile([HID, 128], F32, tag="smallps")
                nc.tensor.matmul(hp[:], cW1[:], combT[:], start=True, stop=True)
                hT = sb.tile([HID, 128], F32, tag="hT")
                nc.scalar.activation(hT[:], hp[:], ACT.Relu, bias=cb1[:])
                lp = ps.tile([NCLS, 128], F32, tag="smallps")
                nc.tensor.matmul(lp[:], cW2[:], hT[:], start=True, stop=True)
                lT = sb.tile([NCLS, 128], F32, tag="lT")
                nc.vector.tensor_scalar_add(lT[:], lp[:], cb2[:])
                ltp = ps.tile([128, NCLS], F32, tag="smallps")
                nc.tensor.transpose(ltp[:], lT[:], ident[:NCLS, :NCLS])
                lg = sb.tile([128, NCLS], F32, tag="lg")
                nc.vector.tensor_copy(lg[:], ltp[:])
                mx = sb.tile([128, 1], F32, tag="mx")
                nc.vector.tensor_reduce(mx[:], lg[:], axis=AX.X, op=OP.max)
                sh = sb.tile([128, NCLS], F32, tag="sh")
                nc.vector.tensor_scalar_sub(sh[:], lg[:], mx[:])
                ex = sb.tile([128, NCLS], F32, tag="ex")
                nc.scalar.activation(ex[:], sh[:], ACT.Exp)
                sm = sb.tile([128, 1], F32, tag="sm")
                nc.vector.tensor_reduce(sm[:], ex[:], axis=AX.X, op=OP.add)
                lns = sb.tile([128, 1], F32, tag="lns")
                nc.scalar.activation(lns[:], sm[:], ACT.Ln)
                fin = sb.tile([128, NCLS], F32, tag="fin")
                nc.vector.tensor_scalar_sub(fin[:], sh[:], lns[:])
                nc.sync.dma_start(out=out[gsl, :], in_=fin[:])
    nc.compile()
    return nc


def kernel(x, edge_index, batch, mri, cog, clin, genetic,
           gcn_W, gcn_b, mri_W, mri_b, cog_W, cog_b, clin_W, clin_b,
           gen_W, gen_b, cls_W1, cls_b1, cls_W2, cls_b2):
    x = np.asarray(x, np.float32)
    src = np.asarray(edge_index[0], np.int64)
    dst = np.asarray(edge_index[1], np.int64)
    batch = np.asarray(batch, np.int64)

    deg_in = np.bincount(dst, minlength=NTOT)
    deg = (deg_in + 1.0).astype(np.float32)          # incl. self loop
    dinv = 1.0 / np.sqrt(deg)
    is_real = (np.arange(NTOT) < N_NODES).astype(np.int8)

    # global rank: pads first, then ascending in-degree -> core r%8, pos r//8.
    # All cores see identical degree profiles, so the cross-core-unified
    # per-tile K padding is ~1% instead of ~16%.
    ranked = np.lexsort((deg_in, is_real))
    rank = np.empty(NTOT, np.int64)
    rank[ranked] = np.arange(NTOT)
    core_of = rank % NCORE
    posc = rank // NCORE
    gpos = core_of * NLOC + posc                     # table row of each node
    squad_all = (gpos[src] // 4).astype(np.int16)
    ssub_all = gpos[src] % 4
    dcore = core_of[dst]
    dl_all = posc[dst]

    Ks = np.ones(NT, np.int64)
    cnts_by_core = []
    for c in range(NCORE):
        sel = dcore == c
        cnts = np.bincount(dl_all[sel], minlength=NLOC)
        cnts_by_core.append(cnts)
        Ks = np.maximum(Ks, cnts.reshape(NT, 128).max(axis=1))
    Ks = [int(k) for k in Ks]
    cum8 = np.zeros(NT + 1, np.int64)
    np.cumsum([8 * k for k in Ks], out=cum8[1:])
    cum4 = np.zeros(NT + 1, np.int64)
    np.cumsum([4 * k for k in Ks], out=cum4[1:])
    totw, tot4 = int(cum8[-1]), int(cum4[-1])

    giota = np.tile(np.arange(NGRAPH, dtype=np.float32), (128, 1))
    in_maps = []
    for c in range(NCORE):
        sel = np.nonzero(dcore == c)[0]
        dl = dl_all[sel]
        order = np.argsort(dl, kind='stable')
        dl_s = dl[order]
        sq_s = squad_all[sel][order]
        ss_s = ssub_all[sel][order]
        cnts = cnts_by_core[c]
        starts = np.zeros(NLOC + 1, np.int64)
        np.cumsum(cnts, out=starts[1:])
        kk = np.arange(len(dl_s)) - starts[dl_s]
        tt = dl_s // 128
        ppart = dl_s % 128

        j = kk * 128 + ppart
        slotq16 = np.zeros((16, totw), np.int16)
        slotq16[j % 16, cum8[tt] + j // 16] = sq_s
        slotq = np.tile(slotq16, (8, 1))
        mskv = np.zeros((128, tot4), np.float32)
        mskv[ppart, cum4[tt] + kk * 4 + ss_s] = dinv[dst[sel][order]]

        node_at = ranked[c::NCORE]
        realm = node_at < N_NODES
        xs = np.zeros((NLOC, NODE_DIM), np.float32)
        xs[realm] = x[node_at[realm]]
        gid = np.where(realm, batch[np.minimum(node_at, N_NODES - 1)],
                       -1).astype(np.float32)
        m = {
            'x16': xs.astype(ml_dtypes.bfloat16),
            'W16': np.asarray(gcn_W, np.float32).astype(ml_dtypes.bfloat16),
            'degT': deg[node_at].reshape(NT, 128).T.copy(),
            'gidxT': gid.reshape(NT, 128).T.astype(np.float32).copy(),
            'giota': giota,
            'brep': np.tile(np.asarray(gcn_b, np.float32), (128, 1)),
            'slotq': slotq, 'mskv': mskv,
            'cW1': np.asarray(cls_W1, np.float32),
            'cb1': np.asarray(cls_b1, np.float32).reshape(-1, 1),
            'cW2': np.asarray(cls_W2, np.float32),
            'cb2': np.asarray(cls_b2, np.float32).reshape(-1, 1),
        }
        for (mn, fdim), mv, wv, bv in zip(
                MODS, (mri, cog, clin, genetic),
                (mri_W, cog_W, clin_W, gen_W), (mri_b, cog_b, clin_b, gen_b)):
            m[mn + 'T'] = np.asarray(mv, np.float32).T.copy()
            m[mn + 'W'] = np.asarray(wv, np.float32)
            m[mn + 'b'] = np.asarray(bv, np.float32).reshape(-1, 1)
        in_maps.append(m)

    nc = _build(Ks, totw, tot4)
    res = run_bass_kernel_spmd(nc, in_maps, core_ids=list(range(NCORE)))
    return res.results[0]["out"].astype(np.float32)


# revision 10
# speedup vs baseline: 1.2892x; 1.0477x over previous
import sys
if '/opt/trn_rl_repo' not in sys.path:
    sys.path.insert(0, '/opt/trn_rl_repo')
import numpy as np
import ml_dtypes

import concourse.bass as bass
import concourse.bacc as bacc
import concourse.mybir as mybir
import concourse.tile as tile
from concourse import library_config
from concourse.masks import make_identity
from concourse.bass_utils import run_bass_kernel_spmd
from concourse._compat import cdiv

NCORE = 8
N_NODES = 100000
N_EDGES = 3200000
NODE_DIM = 128
HID = 16
NGRAPH = 256
NCLS = 3
NLOC = 12544            # nodes per core (98 tiles of 128); 8*12544 = 100352
NT = NLOC // 128        # 98 tiles per core
NTOT = NCORE * NLOC
MODS = [("mri", 256), ("cog", 64), ("clin", 32), ("gen", 512)]

F32 = mybir.dt.float32
BF16 = mybir.dt.bfloat16
I16 = mybir.dt.int16
AX = mybir.AxisListType
OP = mybir.AluOpType
ACT = mybir.ActivationFunctionType


def _build(Ks, totw, tot4):
    """Per-core SPMD program; same pipeline shape as the proven baseline,
    with the sub-slot select folded into a host-built mask (msk carries
    dinv[dst] at the live sub-slot, 0 elsewhere)."""
    nc = bacc.Bacc(num_swdge_queues=4)
    P = {}
    P['x16'] = nc.declare_dram_parameter("x16", [NLOC, NODE_DIM], BF16, isOutput=False)
    P['W16'] = nc.declare_dram_parameter("W16", [NODE_DIM, HID], BF16, isOutput=False)
    P['degT'] = nc.declare_dram_parameter("degT", [128, NT], F32, isOutput=False)
    P['gidxT'] = nc.declare_dram_parameter("gidxT", [128, NT], F32, isOutput=False)
    P['giota'] = nc.declare_dram_parameter("giota", [128, NGRAPH], F32, isOutput=False)
    P['brep'] = nc.declare_dram_parameter("brep", [128, HID], F32, isOutput=False)
    P['slotq'] = nc.declare_dram_parameter("slotq", [128, totw], I16, isOutput=False)
    P['mskv'] = nc.declare_dram_parameter("mskv", [128, tot4], F32, isOutput=False)
    for m, fdim in MODS:
        P[m + 'T'] = nc.declare_dram_parameter(m + 'T', [fdim, NGRAPH], F32, isOutput=False)
        P[m + 'W'] = nc.declare_dram_parameter(m + 'W', [fdim, 4], F32, isOutput=False)
        P[m + 'b'] = nc.declare_dram_parameter(m + 'b', [4, 1], F32, isOutput=False)
    P['cW1'] = nc.declare_dram_parameter("cW1", [32, HID], F32, isOutput=False)
    P['cb1'] = nc.declare_dram_parameter("cb1", [HID, 1], F32, isOutput=False)
    P['cW2'] = nc.declare_dram_parameter("cW2", [HID, NCLS], F32, isOutput=False)
    P['cb2'] = nc.declare_dram_parameter("cb2", [NCLS, 1], F32, isOutput=False)
    out = nc.declare_dram_parameter("out", [NGRAPH, NCLS], F32, isOutput=True)

    tloc = nc.dram_tensor("tloc", [NLOC, HID], F32)
    table = nc.dram_tensor("table", [NTOT, HID], F32, addr_space="Shared")
    poolloc = nc.dram_tensor("poolloc", [HID + 1, NGRAPH], F32)
    poolred = nc.dram_tensor("poolred", [HID + 1, NGRAPH], F32, addr_space="Shared")
    groups = [list(range(NCORE))]

    with tile.TileContext(nc) as tc:
        with tc.tile_pool(name="pers", bufs=1) as pp, \
             tc.tile_pool(name="sb", bufs=2) as sb, \
             tc.tile_pool(name="gat", bufs=6) as gb, \
             tc.tile_pool(name="ps", bufs=2, space="PSUM") as ps, \
             tc.tile_pool(name="pool_ps", bufs=1, space="PSUM") as pps:
            nc.gpsimd.load_library(library_config.mlp)

            # ---------- phase 1: local xw2 shard + AllGather table ----------
            xT = pp.tile([128, NLOC], BF16)
            nc.sync.dma_start_transpose(xT[:], P['x16'][:])
            Wt = pp.tile([NODE_DIM, HID], BF16)
            nc.sync.dma_start(out=Wt[:], in_=P['W16'][:])
            degT = pp.tile([128, NT], F32)
            nc.sync.dma_start(out=degT[:], in_=P['degT'][:])
            gidxT = pp.tile([128, NT], F32)
            nc.sync.dma_start(out=gidxT[:], in_=P['gidxT'][:])
            giota = pp.tile([128, NGRAPH], F32)
            nc.sync.dma_start(out=giota[:], in_=P['giota'][:])
            brep = pp.tile([128, HID], F32)
            nc.sync.dma_start(out=brep[:], in_=P['brep'][:])
            ident = pp.tile([128, 128], F32)
            make_identity(nc, ident[:])
            ones_t = pp.tile([128, 1], BF16)
            nc.vector.memset(ones_t[:], 1.0)

            dinvT = pp.tile([128, NT], F32)
            rcpT = pp.tile([128, NT], F32)
            nc.vector.reciprocal(rcpT[:], degT[:])
            nc.scalar.activation(dinvT[:], rcpT[:], ACT.Sqrt)
            selfb = pp.tile([128, NT * HID], F32)

            for tb in range(0, NT, 4):
                nb = min(4, NT - tb)
                xwp = ps.tile([128, 4 * HID], F32, tag="smallps")
                for q in range(nb):
                    nc.tensor.matmul(xwp[:, q * HID:(q + 1) * HID],
                                     xT[:, (tb + q) * 128:(tb + q + 1) * 128], Wt[:],
                                     start=True, stop=True)
                xw2 = sb.tile([128, 4 * HID], F32, tag="xw2")
                nc.vector.tensor_tensor(
                    out=xw2[:, :nb * HID].rearrange("p (c f) -> p c f", f=HID),
                    in0=xwp[:, :nb * HID].rearrange("p (c f) -> p c f", f=HID),
                    in1=dinvT[:, tb:tb + nb].unsqueeze(2).to_broadcast([128, nb, HID]),
                    op=OP.mult)
                nc.sync.dma_start(
                    out=tloc[tb * 128:(tb + nb) * 128, :].rearrange(
                        "(c p) f -> p c f", p=128),
                    in_=xw2[:, :nb * HID].rearrange("p (c f) -> p c f", f=HID))
                sbt = sb.tile([128, 4 * HID], F32, tag="sbt")
                nc.vector.tensor_tensor(
                    out=sbt[:, :nb * HID].rearrange("p (c f) -> p c f", f=HID),
                    in0=xw2[:, :nb * HID].rearrange("p (c f) -> p c f", f=HID),
                    in1=dinvT[:, tb:tb + nb].unsqueeze(2).to_broadcast([128, nb, HID]),
                    op=OP.mult)
                nc.vector.tensor_tensor(
                    out=selfb[:, tb * HID:(tb + nb) * HID].rearrange(
                        "p (c f) -> p c f", f=HID),
                    in0=sbt[:, :nb * HID].rearrange("p (c f) -> p c f", f=HID),
                    in1=brep[:].unsqueeze(1).to_broadcast([128, nb, HID]),
                    op=OP.add)

            nc.gpsimd.collective_compute(
                "AllGather", OP.bypass, replica_groups=groups,
                ins=[tloc[:]], outs=[table[:]])

            # ---------- phase 2: gather + masked aggregate + pool ----------
            tview = table[:].rearrange("(q s) f -> q (s f)", s=4)
            pool_psum = pps.tile([HID + 1, NGRAPH], F32)
            cum8v = [0]
            cum4v = [0]
            for k in Ks:
                cum8v.append(cum8v[-1] + 8 * k)
                cum4v.append(cum4v[-1] + 4 * k)
            t_order = sorted(range(NT), key=lambda tt: -Ks[tt])
            for pos, t in enumerate(t_order):
                K = Ks[t]
                woff = cum8v[t]
                ooff = cum4v[t]
                it = gb.tile([128, 8 * max(Ks)], I16, tag="it")
                nc.sync.dma_start(out=it[:, :8 * K], in_=P['slotq'][:, woff:woff + 8 * K])
                mk = gb.tile([128, 4 * max(Ks)], F32, tag="mk")
                nc.sync.dma_start(out=mk[:, :4 * K], in_=P['mskv'][:, ooff:ooff + 4 * K])

                gt = gb.tile([128, max(Ks) * 64], F32, tag="gt")
                nc.gpsimd.dma_gather(
                    gt[:, :K * 64].rearrange("p (c e) -> p c e", e=64),
                    tview, it[:, :8 * K], 128 * K, 128 * K, 64,
                    single_packet=False, queue_num=pos % 4)
                mskd = sb.tile([128, max(Ks) * 64], BF16, tag="mskd")
                nc.vector.tensor_tensor(
                    out=mskd[:, :K * 64].rearrange("p (m f) -> p m f", f=HID),
                    in0=gt[:, :K * 64].rearrange("p (m f) -> p m f", f=HID),
                    in1=mk[:, :4 * K].unsqueeze(2).to_broadcast([128, 4 * K, HID]),
                    op=OP.mult)
                nsum = sb.tile([128, HID], F32, tag="nsum")
                nc.vector.tensor_reduce(
                    nsum[:], mskd[:, :K * 64].rearrange("p (m f) -> p f m", f=HID),
                    axis=AX.X, op=OP.add)

                pre = sb.tile([128, HID], F32, tag="pre")
                nc.vector.tensor_add(pre[:], nsum[:], selfb[:, t * HID:(t + 1) * HID])
                f17 = sb.tile([128, HID + 1], BF16, tag="f17")
                nc.scalar.activation(f17[:, :HID], pre[:], ACT.Relu)
                nc.vector.tensor_copy(f17[:, HID:HID + 1], ones_t[:])
                G = sb.tile([128, NGRAPH], BF16, tag="G")
                nc.vector.tensor_tensor(out=G[:], in0=gidxT[:, t:t + 1].to_broadcast([128, NGRAPH]),
                                        in1=giota[:], op=OP.is_equal)
                nc.tensor.matmul(pool_psum[:], f17[:], G[:],
                                 start=(pos == 0), stop=(pos == NT - 1))

            # ---------- phase 3: all-reduce pooled sums; replicated head ----------
            pool_s = sb.tile([HID + 1, NGRAPH], F32)
            nc.vector.tensor_copy(pool_s[:], pool_psum[:])
            nc.sync.dma_start(out=poolloc[:], in_=pool_s[:])
            nc.gpsimd.collective_compute(
                "AllReduce", OP.add, replica_groups=groups,
                ins=[poolloc[:]], outs=[poolred[:]])
            pool_r = pp.tile([HID + 1, NGRAPH], F32)
            nc.sync.dma_start(out=pool_r[:], in_=poolred[:])

            # modality MLPs -> mT tiles [4, 256] in sbuf
            mod_sb = {}
            for m, fdim in MODS:
                mt = pp.tile([fdim if fdim <= 128 else 128,
                              NGRAPH * cdiv(fdim, 128)], F32, tag="mt_" + m)
                if fdim <= 128:
                    nc.sync.dma_start(out=mt[:fdim, :NGRAPH], in_=P[m + 'T'][:])
                else:
                    for k in range(fdim // 128):
                        nc.sync.dma_start(out=mt[:, k * NGRAPH:(k + 1) * NGRAPH],
                                          in_=P[m + 'T'][k * 128:(k + 1) * 128, :])
                wt = pp.tile([fdim if fdim <= 128 else 128,
                              4 * cdiv(fdim, 128)], F32, tag="mw_" + m)
                if fdim <= 128:
                    nc.sync.dma_start(out=wt[:fdim, :4], in_=P[m + 'W'][:])
                else:
                    for k in range(fdim // 128):
                        nc.sync.dma_start(out=wt[:, k * 4:(k + 1) * 4],
                                          in_=P[m + 'W'][k * 128:(k + 1) * 128, :])
                bt = pp.tile([4, 1], F32, tag="mb_" + m)
                nc.sync.dma_start(out=bt[:], in_=P[m + 'b'][:])
                mp = ps.tile([4, NGRAPH], F32, tag="smallps")
                nk = cdiv(fdim, 128)
                for k in range(nk):
                    kk = min(128, fdim - k * 128)
                    nc.tensor.matmul(mp[:], wt[:kk, k * 4:k * 4 + 4],
                                     mt[:kk, k * NGRAPH:(k + 1) * NGRAPH],
                                     start=(k == 0), stop=(k == nk - 1))
                msb = pp.tile([4, NGRAPH], F32, tag="msb_" + m)
                nc.scalar.activation(msb[:], mp[:], ACT.Relu, bias=bt[:])
                mod_sb[m] = msb

            cW1 = pp.tile([32, HID], F32)
            nc.sync.dma_start(out=cW1[:], in_=P['cW1'][:])
            cb1 = pp.tile([HID, 1], F32)
            nc.sync.dma_start(out=cb1[:], in_=P['cb1'][:])
            cW2 = pp.tile([HID, NCLS], F32)
            nc.sync.dma_start(out=cW2[:], in_=P['cW2'][:])
            cb2 = pp.tile([NCLS, 1], F32)
            nc.sync.dma_start(out=cb2[:], in_=P['cb2'][:])

            for gtile in range(2):
                gsl = slice(gtile * 128, (gtile + 1) * 128)
                # transpose pooled sums -> [128 graphs, 17]
                tp = ps.tile([128, HID + 1], F32, tag="smallps")
                nc.tensor.transpose(tp[:], pool_r[:, gsl], ident[:HID + 1, :HID + 1])
                gsum = sb.tile([128, HID + 1], F32, tag="gsum")
                nc.vector.tensor_copy(gsum[:], tp[:])
                cnt = sb.tile([128, 1], F32, tag="cnt")
                nc.vector.tensor_scalar_max(cnt[:], gsum[:, HID:HID + 1], 1.0)
                rec = sb.tile([128, 1], F32, tag="rec")
                nc.vector.reciprocal(rec[:], cnt[:])
                comb = sb.tile([128, 32], F32, tag="comb")
                nc.vector.tensor_scalar_mul(comb[:, :HID], gsum[:, :HID], rec[:])
                coff = HID
                for m, fdim in MODS:
                    mtp = ps.tile([128, 4], F32, tag="smallps")
                    nc.tensor.transpose(mtp[:], mod_sb[m][:, gsl], ident[:4, :4])
                    nc.vector.tensor_copy(comb[:, coff:coff + 4], mtp[:])
                    coff += 4
                # classifier
                ctp = ps.tile([32, 128], F32, tag="smallps")
                nc.tensor.transpose(ctp[:], comb[:], ident[:])
                combT = sb.tile([32, 128], F32, tag="combT")
                nc.vector.tensor_copy(combT[:], ctp[:])
                hp = ps.tile([HID, 128], F32, tag="smallps")
                nc.tensor.matmul(hp[:], cW1[:], combT[:], start=True, stop=True)
                hT = sb.tile([HID, 128], F32, tag="hT")
                nc.scalar.activation(hT[:], hp[:], ACT.Relu, bias=cb1[:])
                lp = ps.tile([NCLS, 128], F32, tag="smallps")
                nc.tensor.matmul(lp[:], cW2[:], hT[:], start=True, stop=True)
                lT = sb.tile([NCLS, 128], F32, tag="lT")
                nc.vector.tensor_scalar_add(lT[:], lp[:], cb2[:])
                ltp = ps.tile([128, NCLS], F32, tag="smallps")
                nc.tensor.transpose(ltp[:], lT[:], ident[:NCLS, :NCLS])
                lg = sb.tile([128, NCLS], F32, tag="lg")
                nc.vector.tensor_copy(lg[:], ltp[:])
                mx = sb.tile([128, 1], F32, tag="mx")
                nc.vector.tensor_reduce(mx[:], lg[:], axis=AX.X, op=OP.max)
                sh = sb.tile([128, NCLS], F32, tag="sh")
                nc.vector.tensor_scalar_sub(sh[:], lg[:], mx[:])
                ex = sb.tile([128, NCLS], F32, tag="ex")
                nc.scalar.activation(ex[:], sh[:], ACT.Exp)
                sm = sb.tile([128, 1], F32, tag="sm")
                nc.vector.tensor_reduce(sm[:], ex[:], axis=AX.X, op=OP.add)
                lns = sb.tile([128, 1], F32, tag="lns")
                nc.scalar.activation(lns[:], sm[:], ACT.Ln)
                fin = sb.tile([128, NCLS], F32, tag="fin")
                nc.vector.tensor_scalar_sub(fin[:], sh[:], lns[:])
                nc.sync.dma_start(out=out[gsl, :], in_=fin[:])
    nc.compile()
    return nc


def kernel(x, edge_index, batch, mri, cog, clin, genetic,
           gcn_W, gcn_b, mri_W, mri_b, cog_W, cog_b, clin_W, clin_b,
           gen_W, gen_b, cls_W1, cls_b1, cls_W2, cls_b2):
    x = np.asarray(x, np.float32)
    src = np.asarray(edge_index[0], np.int64)
    dst = np.asarray(edge_index[1], np.int64)
    batch = np.asarray(batch, np.int64)

    deg_in = np.bincount(dst, minlength=NTOT)
    deg = (deg_in + 1.0).astype(np.float32)          # incl. self loop
    dinv = 1.0 / np.sqrt(deg)
    is_real = (np.arange(NTOT) < N_NODES).astype(np.int8)

    # global rank: pads first, then ascending in-degree -> core r%8, pos r//8.
    # All cores see identical degree profiles, so the cross-core-unified
    # per-tile K padding is ~1% instead of ~16%.
    ranked = np.lexsort((deg_in, is_real))
    rank = np.empty(NTOT, np.int64)
    rank[ranked] = np.arange(NTOT)
    core_of = rank % NCORE
    posc = rank // NCORE
    gpos = core_of * NLOC + posc                     # table row of each node
    squad_all = (gpos[src] // 4).astype(np.int16)
    ssub_all = gpos[src] % 4
    dcore = core_of[dst]
    dl_all = posc[dst]

    Ks = np.ones(NT, np.int64)
    cnts_by_core = []
    for c in range(NCORE):
        sel = dcore == c
        cnts = np.bincount(dl_all[sel], minlength=NLOC)
        cnts_by_core.append(cnts)
        Ks = np.maximum(Ks, cnts.reshape(NT, 128).max(axis=1))
    Ks = [int(k) for k in Ks]
    cum8 = np.zeros(NT + 1, np.int64)
    np.cumsum([8 * k for k in Ks], out=cum8[1:])
    cum4 = np.zeros(NT + 1, np.int64)
    np.cumsum([4 * k for k in Ks], out=cum4[1:])
    totw, tot4 = int(cum8[-1]), int(cum4[-1])

    giota = np.tile(np.arange(NGRAPH, dtype=np.float32), (128, 1))
    in_maps = []
    for c in range(NCORE):
        sel = np.nonzero(dcore == c)[0]
        dl = dl_all[sel]
        order = np.argsort(dl, kind='stable')
        dl_s = dl[order]
        sq_s = squad_all[sel][order]
        ss_s = ssub_all[sel][order]
        cnts = cnts_by_core[c]
        starts = np.zeros(NLOC + 1, np.int64)
        np.cumsum(cnts, out=starts[1:])
        kk = np.arange(len(dl_s)) - starts[dl_s]
        tt = dl_s // 128
        ppart = dl_s % 128

        j = kk * 128 + ppart
        slotq16 = np.zeros((16, totw), np.int16)
        slotq16[j % 16, cum8[tt] + j // 16] = sq_s
        slotq = np.tile(slotq16, (8, 1))
        mskv = np.zeros((128, tot4), np.float32)
        mskv[ppart, cum4[tt] + kk * 4 + ss_s] = dinv[dst[sel][order]]

        node_at = ranked[c::NCORE]
        realm = node_at < N_NODES
        xs = np.zeros((NLOC, NODE_DIM), np.float32)
        xs[realm] = x[node_at[realm]]
        gid = np.where(realm, batch[np.minimum(node_at, N_NODES - 1)],
                       -1).astype(np.float32)
        m = {
            'x16': xs.astype(ml_dtypes.bfloat16),
            'W16': np.asarray(gcn_W, np.float32).astype(ml_dtypes.bfloat16),
            'degT': deg[node_at].reshape(NT, 128).T.copy(),
            'gidxT': gid.reshape(NT, 128).T.astype(np.float32).copy(),
            'giota': giota,
            'brep': np.tile(np.asarray(gcn_b, np.float32), (128, 1)),
            'slotq': slotq, 'mskv': mskv,
            'cW1': np.asarray(cls_W1, np.float32),
            'cb1': np.asarray(cls_b1, np.float32).reshape(-1, 1),
            'cW2': np.asarray(cls_W2, np.float32),
            'cb2': np.asarray(cls_b2, np.float32).reshape(-1, 1),
        }
        for (mn, fdim), mv, wv, bv in zip(
                MODS, (mri, cog, clin, genetic),
                (mri_W, cog_W, clin_W, gen_W), (mri_b, cog_b, clin_b, gen_b)):
            m[mn + 'T'] = np.asarray(mv, np.float32).T.copy()
            m[mn + 'W'] = np.asarray(wv, np.float32)
            m[mn + 'b'] = np.asarray(bv, np.float32).reshape(-1, 1)
        in_maps.append(m)

    nc = _build(Ks, totw, tot4)
    res = run_bass_kernel_spmd(nc, in_maps, core_ids=list(range(NCORE)))
    return res.results[0]["out"].astype(np.float32)
